# revision 1
# baseline (speedup 1.0000x reference)
"""Trainium2 Bass kernel for nn_MultiHeadAttention_5334349382389 (v2).

Sharding: 8 cores = 4 batches x 2 head-groups (4 heads each).
Core c handles batch b = c // 2, head-group g = c % 2 (heads 4g..4g+3).

Per-core math (fp16 matmuls, fp32 PSUM accumulate):
  qhT = (Wq_g/8) @ x_b^T + bq_g/8        [256, 1024]   (score scale folded into Wq)
  khT = Wk_g @ x_b^T + bk_g              [256, 1024]
  vh  = x_b @ Wv_g^T                     [1024, 256]   (bv folded into host-side bias)
  per head h: scoresT[k,q]; h==0 accumulates I @ edgeT on the PE into the
      score PSUM (edgeT is zeros on non-edge cores; Wq/bq head-0 slice
      zeroed on edge cores, so edge cores get scoresT == edgeT exactly)
  expT = exp(scoresT)                    (no max-subtraction; inputs bounded)
  outT_raw[d,q] accum over k-tiles with lhsT = [vh | ones] -> row 64 = denom
  OT = outT_raw[:64] * bcast(1/denom)
  partial = OT^T-contraction @ WoT_g     [1024, 512]
Host: out[b] = partial(b,0) + partial(b,1) + (bo + Wo @ bv).

Schedule (vs the 104us packed-DMA baseline):
- input DMAs balanced across all three issuing queues (each dma_start holds
  its sequencer for the whole transfer, ~85GB/s per active queue); the
  critical q/k stream is split three ways, v-stream next, edge tiles last;
  the Act queue only carries tiny loads so the exps never queue behind DMAs
- PE clock-ramp junk matmuls run on a memset tile (no DMA dependency) and
  bridge the final normalize chain so the p-state never drops mid-kernel
- head order 1,0,2,3: head 1 carries the v projection (pair-packed PSUM
  tiles, one copy per two k-tiles); head 0 carries the I @ edgeT PSUM
  accumulation on the PE, keeping its DVE free for the deferred k-ch1
  biases and head 1's normalize (hooked into the loop); each head's
  normalize is deferred into the next head's DVE slack
- PV matmuls software-pipelined one k-tile behind the score matmuls so the
  PE never waits on the Act-engine exp (the Act engine is the head-phase
  pacer at ~1.16us per k-tile)
- per-m-tile output stores issued as soon as each out-proj tile is copied
"""

import os
import sys

sys.path.insert(0, "/opt/trn_rl_repo")

import numpy as np

B, SEQ, DIN, DO = 4, 1024, 512, 512
NH_ALL, DK = 8, 64
NHC = 4            # heads per core
DH = NHC * DK      # 256 per-core projected dims
P = 128
CD = DIN // P      # 4 contraction chunks for projections
CH = DH // P       # 2 dh chunks
KT = SEQ // P      # 8 k-tiles
STR = 512          # q-stripe (matmul free dim)
NS = SEQ // STR    # 2 stripes
TVW = NHC * (DK + 1) + DK - 1  # 323: per-k-tile aux width (4x65 + 63 pad)

NJUNK0 = 45        # initial clock-ramp junk matmuls
NJUNK_BRIDGE = 24  # junk bridging the final normalize chain before out-proj

COMPUTE = os.environ.get("KERNEL_COMPUTE_DT", "fp16")  # fp16 | bf16 | fp32r

_nc = None


def _np_dt():
    import ml_dtypes

    return {
        "fp16": np.float16,
        "bf16": ml_dtypes.bfloat16,
        "fp32r": np.float32,
    }[COMPUTE]


def _build():
    global _nc
    if _nc is not None:
        return _nc
    import concourse.bacc as bacc
    import concourse.bass as bass
    import concourse.mybir as mybir
    import concourse.tile as tile

    f32 = mybir.dt.float32
    f32r = mybir.dt.float32r
    cdt = {
        "fp16": mybir.dt.float16,
        "bf16": mybir.dt.bfloat16,
        "fp32r": f32r,
    }[COMPUTE]
    Exp = mybir.ActivationFunctionType.Exp

    nc = bacc.Bacc("TRN2", target_bir_lowering=False, debug=False)

    eye_d = nc.dram_tensor("eye", (P, P), cdt, kind="ExternalInput")
    wq_d = nc.dram_tensor("wq", (P, CD * DH), cdt, kind="ExternalInput")
    wk_d = nc.dram_tensor("wk", (P, CD * DH), cdt, kind="ExternalInput")
    wv_d = nc.dram_tensor("wv", (P, CD * DH), cdt, kind="ExternalInput")
    wo_d = nc.dram_tensor("wo", (P, CH * DO), cdt, kind="ExternalInput")
    xq_d = nc.dram_tensor("xq", (P, CD * SEQ), cdt, kind="ExternalInput")
    xk_d = nc.dram_tensor("xk", (P, CD * SEQ), cdt, kind="ExternalInput")
    xv_d = nc.dram_tensor("xv", (P, KT * CD * P), cdt, kind="ExternalInput")
    bqk = nc.dram_tensor("bqk", (2 * DH, 1), f32, kind="ExternalInput")
    edge = nc.dram_tensor("edge", (SEQ, SEQ), cdt, kind="ExternalInput")
    outp = nc.dram_tensor("outp", (SEQ, DO), cdt, kind="ExternalOutput")

    edge_r = edge.rearrange("(t p) n -> t p n", p=P)
    xq_r = xq_d.rearrange("p (c n) -> p c n", n=SEQ)
    xk_r = xk_d.rearrange("p (c n) -> p c n", n=SEQ)
    xv_r = xv_d.rearrange("p (t w) -> p t w", w=CD * P)
    out_r = outp.rearrange("(t p) n -> p t n", p=P)

    def sl(s):
        return slice(s * STR, (s + 1) * STR)

    with tile.TileContext(nc) as tc:
        with (
            tc.tile_pool(name="inp", bufs=1) as inp,
            tc.tile_pool(name="wts", bufs=1) as wts,
            tc.tile_pool(name="qkp", bufs=1) as qkp,
            tc.tile_pool(name="vhap", bufs=1) as vhap,
            tc.tile_pool(name="expp", bufs=8) as expp,
            tc.tile_pool(name="otp", bufs=1) as otp,
            tc.tile_pool(name="rrp", bufs=4) as rrp,
            tc.tile_pool(name="rbp", bufs=4) as rbp,
            tc.tile_pool(name="oalp", bufs=3) as oalp,
            tc.tile_pool(name="edgp", bufs=8) as edgp,
            tc.tile_pool(name="bigp", bufs=2, space=bass.MemorySpace.PSUM) as bigp,
            tc.tile_pool(name="pvp", bufs=3, space=bass.MemorySpace.PSUM) as pvp,
            tc.tile_pool(name="vpp", bufs=1, space=bass.MemorySpace.PSUM) as vpp,
        ):
            # ---------------- tiles ----------------
            tjk = wts.tile([P, STR], cdt, tag="tjk")
            twq = wts.tile([P, CD, DH], cdt, tag="twq")
            twk = wts.tile([P, CD, DH], cdt, tag="twk")
            twv = wts.tile([P, CD, DH], cdt, tag="twv")
            two = wts.tile([P, CH, DO], cdt, tag="two")
            tb4 = wts.tile([P, 4, 1], f32, tag="tb4")
            txq = inp.tile([P, CD, SEQ], cdt, tag="txq")
            txk = inp.tile([P, CD, SEQ], cdt, tag="txk")
            txv = inp.tile([P, KT, CD, P], cdt, tag="txv")
            tqh = qkp.tile([P, CH, SEQ], cdt, tag="tqh")
            khp = qkp.tile([P, NHC, SEQ], cdt, tag="khp")
            tvha = vhap.tile([P, KT, TVW], cdt, tag="tvha")
            tot = otp.tile([P, CH, SEQ], cdt, tag="tot")

            # ------- memsets: tjk on Pool (first op, gates junk); the rest on
            # DVE so the Pool queue can start issuing DMAs immediately -------
            nc.gpsimd.memset(tjk, 0.0)
            # zero the unused partition-halves of khp (even heads: parts
            # 64-127, odd heads: parts 0-63) so score matmuls see zero weights
            nc.vector.memset(khp[0:DK, 1::2, :], 0.0)
            nc.vector.memset(khp[DK:P, 0::2, :], 0.0)
            # vh-aug tail pad + per-head ones columns (denominator rows)
            nc.vector.memset(tvha[:, :, NHC * (DK + 1) : TVW], 0.0)
            nc.vector.memset(
                tvha[:, :, 0 : NHC * (DK + 1)].rearrange(
                    "p t (h w) -> p t h w", w=DK + 1
                )[:, :, :, DK : DK + 1],
                1.0,
            )

            # ------- input DMAs: each dma_start holds its sequencer through
            # the whole transfer, so use FEW, BIG DMAs on sync + gpsimd (the
            # scalar queue stays clear — exps would queue behind it).  tb4 is
            # tiny and rides ahead of everything on scalar. ------
            ed_pairs = [
                edgp.tile([P, 2, SEQ], cdt, tag="edg", name=f"edp{i}")
                for i in range(KT // 2)
            ]
            edge_r2 = edge.rearrange("(t x p) n -> t p x n", x=2, p=P)
            xv_r4 = xv_d.rearrange("p (t c j) -> p t c j", c=CD, j=P)
            teye = wts.tile([P, P], cdt, tag="teye")
            # queues pull the rings at ~equal rates, so BALANCE the critical
            # q/k stream across all three queues; v-stream next (first PVs
            # need it ~4us after the first score); edges trail (head 0 runs
            # second); wo last.
            xq_r2 = xq_d.rearrange("p (h n) -> p h n", h=2)
            xk_r2 = xk_d.rearrange("p (h n) -> p h n", h=2)
            # queues pull the rings at ~equal rates, so BALANCE the critical
            # q/k stream across all three queues; v-stream next (first PVs
            # need it ~4us after the first score); edges trail (head 0 runs
            # second); wo last.
            nc.scalar.dma_start(out=tb4, in_=bqk.rearrange("(c p) o -> p c o", p=P))
            nc.scalar.dma_start(out=teye, in_=eye_d[:, :])
            nc.scalar.dma_start(out=txq[:, 0:2], in_=xq_r2[:, 0])
            nc.scalar.dma_start(out=txq[:, 2:4], in_=xq_r2[:, 1])
            nc.sync.dma_start(out=twk, in_=wk_d.rearrange("p (c d) -> p c d", d=DH))
            nc.sync.dma_start(out=txk[:, 0:2], in_=xk_r2[:, 0])
            nc.gpsimd.dma_start(
                out=twq, in_=wq_d.rearrange("p (c d) -> p c d", d=DH)
            )
            nc.gpsimd.dma_start(out=txk[:, 2:4], in_=xk_r2[:, 1])
            nc.gpsimd.dma_start(
                out=twv, in_=wv_d.rearrange("p (c d) -> p c d", d=DH)
            )
            nc.gpsimd.dma_start(out=txv[:, 0:4], in_=xv_r4[:, 0:4])
            nc.sync.dma_start(out=ed_pairs[0], in_=edge_r2[0])
            nc.sync.dma_start(out=ed_pairs[1], in_=edge_r2[1])
            nc.gpsimd.dma_start(out=txv[:, 4:KT], in_=xv_r4[:, 4:KT])
            nc.gpsimd.dma_start(
                out=two, in_=wo_d.rearrange("p (c d) -> p c d", d=DO)
            )
            nc.sync.dma_start(out=ed_pairs[2], in_=edge_r2[2])
            nc.sync.dma_start(out=ed_pairs[3], in_=edge_r2[3])

            # PE clock-ramp filler: junk matmuls on the memset tile keep the
            # p-state ramp going while DMAs land. Shares the vpp PSUM bank
            # (vproj runs much later); output never read.
            def junk(n):
                jt = vpp.tile([P, STR], f32, tag="vp")
                for _ in range(n):
                    nc.tensor.matmul(
                        jt[:], lhsT=tjk[:, 0:P], rhs=tjk[:], start=True, stop=True
                    )

            junk(NJUNK0)

            # ------- projections upfront: q-ch0, q-ch1 (bigp) and k-ch0
            # stripes (pvp banks, idle during the proj phase).  Only k-ch1
            # stays behind as a small block after head 1. -------
            ptqa = bigp.tile([P, SEQ], f32, tag="big")
            ptqb = bigp.tile([P, SEQ], f32, tag="big")
            kc0s = [
                pvp.tile([P, STR], f32, tag="pv", name=f"kc0s{s}") for s in range(NS)
            ]
            for cd in range(CD):
                for s in range(NS):
                    nc.tensor.matmul(
                        kc0s[s][:],
                        lhsT=twk[:, cd, 0:P],
                        rhs=txk[:, cd, sl(s)],
                        start=(cd == 0),
                        stop=(cd == CD - 1),
                    )
                for s in range(NS):
                    nc.tensor.matmul(
                        ptqa[:, sl(s)],
                        lhsT=twq[:, cd, 0:P],
                        rhs=txq[:, cd, sl(s)],
                        start=(cd == 0),
                        stop=(cd == CD - 1),
                    )
                for s in range(NS):
                    nc.tensor.matmul(
                        ptqb[:, sl(s)],
                        lhsT=twq[:, cd, P : 2 * P],
                        rhs=txq[:, cd, sl(s)],
                        start=(cd == 0),
                        stop=(cd == CD - 1),
                    )
            # biases on DVE, by gate: head 1 first (tqh-ch0 + khp-odd), then
            # releases for the stt ring (ptqa/ptqb) and pvp slots (kc0s)
            nc.vector.tensor_scalar_add(
                out=tqh[:, 0, sl(0)], in0=ptqa[:, sl(0)], scalar1=tb4[:, 0, :]
            )
            nc.vector.tensor_scalar_add(
                out=khp[DK:P, 1, sl(0)], in0=kc0s[0][DK:P, :], scalar1=tb4[DK:P, 2, :]
            )
            nc.vector.tensor_scalar_add(
                out=tqh[:, 0, sl(1)], in0=ptqa[:, sl(1)], scalar1=tb4[:, 0, :]
            )
            nc.vector.tensor_scalar_add(
                out=khp[DK:P, 1, sl(1)], in0=kc0s[1][DK:P, :], scalar1=tb4[DK:P, 2, :]
            )
            nc.vector.tensor_scalar_add(
                out=tqh[:, 1, :], in0=ptqb[:], scalar1=tb4[:, 1, :]
            )
            nc.vector.tensor_scalar_add(
                out=khp[0:DK, 0, sl(0)], in0=kc0s[0][0:DK, :], scalar1=tb4[0:DK, 2, :]
            )
            nc.vector.tensor_scalar_add(
                out=khp[0:DK, 0, sl(1)], in0=kc0s[1][0:DK, :], scalar1=tb4[0:DK, 2, :]
            )

            def proj_kch1_mms():
                # k-ch1 block after head 1 (its bigp slot is released by the
                # khp-ch1 biases hooked early into head 0)
                ptk2 = bigp.tile([P, SEQ], f32, tag="big")
                for cd in range(CD):
                    for s in range(NS):
                        nc.tensor.matmul(
                            ptk2[:, sl(s)],
                            lhsT=twk[:, cd, P : 2 * P],
                            rhs=txk[:, cd, sl(s)],
                            start=(cd == 0),
                            stop=(cd == CD - 1),
                        )
                return ptk2

            # ---------------- attention per head ----------------
            # processing order: h1 (carries the v projection), h0 (edge adds
            # on DVE), h2, h3.  PV matmuls run one k-tile behind the scores.
            def head_body(h, with_vproj=False, extra_dve=None):
                # extra_dve: {kt: closure} — deferred DVE ops (ch1 proj
                # biases) emitted into this head's DVE slack after tile kt
                ch, off = h // 2, (h % 2) * DK
                edge_h = h == 0
                pv0 = pvp.tile([P, STR], f32, tag="pv")
                pv1 = pvp.tile([P, STR], f32, tag="pv")
                pvs = (pv0, pv1)

                def pv_mm(lte, lkt, stop):
                    for s in range(NS):
                        nc.tensor.matmul(
                            pvs[s][:],
                            lhsT=tvha[:, lkt, h * (DK + 1) : h * (DK + 1) + P],
                            rhs=lte[:, sl(s)],
                            start=(lkt == 0),
                            stop=stop,
                        )

                lag = None
                for kt in range(KT):
                    stt = bigp.tile([P, SEQ], f32, tag="big")
                    for s in range(NS):
                        nc.tensor.matmul(
                            stt[:, sl(s)],
                            lhsT=khp[:, h, kt * P : (kt + 1) * P],
                            rhs=tqh[:, ch, sl(s)],
                            start=True,
                            stop=(not edge_h),
                        )
                        if edge_h:
                            # edge injection on the PE: accumulate I @ edgeT
                            # (keeps the DVE free for the deferred bias/norm)
                            nc.tensor.matmul(
                                stt[:, sl(s)],
                                lhsT=teye[:],
                                rhs=ed_pairs[kt // 2][:, kt % 2, sl(s)],
                                start=False,
                                stop=True,
                            )
                    if with_vproj and kt % 2 == 0:
                        # v projection for k-tiles kt, kt+1 packed into one
                        # PSUM tile + one copy (halves the vpp ring traffic)
                        vp = vpp.tile([P, STR], f32, tag="vp")
                        for u in range(2):
                            for cd in range(CD):
                                nc.tensor.matmul(
                                    vp[:, u * DH : (u + 1) * DH],
                                    lhsT=txv[:, kt + u, cd, :],
                                    rhs=twv[:, cd, :],
                                    start=(cd == 0),
                                    stop=(cd == CD - 1),
                                )
                        nc.vector.tensor_copy(
                            out=tvha[:, kt : kt + 2, 0 : NHC * (DK + 1)].rearrange(
                                "p t (h w) -> p t h w", w=DK + 1
                            )[:, :, :, 0:DK],
                            in_=vp[:].rearrange("p (t h d) -> p t h d", t=2, h=NHC),
                        )
                    te = expp.tile([P, SEQ], cdt, tag="expT")
                    nc.scalar.activation(out=te, in_=stt[:], func=Exp)
                    if extra_dve and kt in extra_dve:
                        extra_dve[kt]()
                    if lag is not None:
                        pv_mm(lag[0], lag[1], stop=False)
                    lag = (te, kt)
                pv_mm(lag[0], lag[1], stop=True)
                return pvs

            def norm_recip(pvs, s):
                # denominator row -> SBUF -> approx reciprocal (DVE)
                rr = rrp.tile([1, STR], f32, tag="rr")
                rs = rrp.tile([1, STR], f32, tag="rs")
                nc.vector.tensor_copy(out=rs[:], in_=pvs[s][DK : DK + 1, :])
                nc.vector.reciprocal_approx_fast(out=rr[:], in_=rs[:])
                return rr

            def norm_mul(h, pvs, s, rr):
                ch, off = h // 2, (h % 2) * DK
                rb = rbp.tile([DK, STR], f32, tag="rb")
                nc.gpsimd.partition_broadcast(rb[:], rr[:])
                nc.vector.tensor_mul(
                    tot[off : off + DK, ch, sl(s)], pvs[s][0:DK, :], rb[:]
                )

            def normalize(h, pvs):
                for s in range(NS):
                    norm_mul(h, pvs, s, norm_recip(pvs, s))

            pvs1 = head_body(1, with_vproj=True)
            ptk2 = proj_kch1_mms()

            # head 0's DVE is free (edge rides the PE), so the k-ch1 biases
            # and head 1's normalize dribble into its hooks, ordered so the
            # bigp slot (khp biases) and pvp slots (h1 norm) release early.
            rrs = {}

            def hooks_for(prev_h, prev_pvs, pre=None):
                def hk(s_ops):
                    def run():
                        for op in s_ops:
                            op()
                    return run
                mk = {}
                mk[0] = hk(
                    (pre or [])
                    + [lambda: rrs.__setitem__(prev_h * 2, norm_recip(prev_pvs, 0))]
                )
                mk[1] = hk([lambda: norm_mul(prev_h, prev_pvs, 0, rrs[prev_h * 2])])
                mk[2] = hk(
                    [lambda: rrs.__setitem__(prev_h * 2 + 1, norm_recip(prev_pvs, 1))]
                )
                mk[3] = hk([lambda: norm_mul(prev_h, prev_pvs, 1, rrs[prev_h * 2 + 1])])
                return mk

            h0_hooks = hooks_for(
                1,
                pvs1,
                pre=[
                    lambda: nc.vector.tensor_scalar_add(
                        out=khp[0:DK, 2, :],
                        in0=ptk2[0:DK, :],
                        scalar1=tb4[0:DK, 3, :],
                    ),
                    lambda: nc.vector.tensor_scalar_add(
                        out=khp[DK:P, 3, :],
                        in0=ptk2[DK:P, :],
                        scalar1=tb4[DK:P, 3, :],
                    ),
                ],
            )
            pvs0 = head_body(0, extra_dve=h0_hooks)
            pvs2 = head_body(2, extra_dve=hooks_for(0, pvs0))
            pvs3 = head_body(3, extra_dve=hooks_for(2, pvs2))

            # tail: junk bridge holds the PE clock while the last head's
            # normalize chain (Act copy -> DVE recip -> PE broadcast -> DVE
            # mul) resolves
            junk(NJUNK_BRIDGE)
            for s in range(NS):
                rs = rrp.tile([1, STR], f32, tag="rs")
                nc.scalar.copy(out=rs[:], in_=pvs3[s][DK : DK + 1, :])
                rr = rrp.tile([1, STR], f32, tag="rr")
                nc.vector.reciprocal_approx_fast(out=rr[:], in_=rs[:])
                rb = rbp.tile([DK, STR], f32, tag="rb")
                nc.gpsimd.partition_broadcast(rb[:], rr[:])
                nc.vector.tensor_mul(
                    tot[DK : 2 * DK, 1, sl(s)], pvs3[s][0:DK, :], rb[:]
                )

            # ---------------- output projection ----------------
            store_eng = [
                nc.sync, nc.gpsimd, nc.sync, nc.gpsimd,
                nc.sync, nc.gpsimd, nc.scalar, nc.scalar,
            ]
            for m in range(KT):
                po = bigp.tile([P, SEQ], f32, tag="big")
                for ci in range(CH):
                    nc.tensor.matmul(
                        po[:, 0:DO],
                        lhsT=tot[:, ci, m * P : (m + 1) * P],
                        rhs=two[:, ci, :],
                        start=(ci == 0),
                        stop=(ci == CH - 1),
                    )
                oal = oalp.tile([P, DO], cdt, tag="oall")
                nc.vector.tensor_copy(out=oal[:], in_=po[:, 0:DO])
                store_eng[m].dma_start(out=out_r[:, m], in_=oal[:])

    nc.compile()
    _nc = nc
    return nc


def _in_maps(q, k, v, edge_matrix, Wq, bq, Wk, bk, Wv, Wo):
    dt = _np_dt()
    zeros_edge = np.zeros((SEQ, SEQ), dt)
    edge_t = np.ascontiguousarray(edge_matrix.T).astype(dt)

    def re_cp(m):
        # [C*P, D] -> [P, C*D] (partition-major packing of "(c p) d -> p c d")
        cp, d = m.shape
        return np.ascontiguousarray(
            m.reshape(cp // P, P, d).transpose(1, 0, 2).reshape(P, -1)
        )

    def re_st(m):
        # [CD*P, KT*P] -> [P, KT*CD*P]: st-major packing for the v stream
        return np.ascontiguousarray(
            m.reshape(CD, P, KT, P).transpose(1, 2, 0, 3).reshape(P, -1)
        )

    xt = {}
    for b in range(B):
        xt[b] = (
            re_cp(np.ascontiguousarray(q[b].T).astype(dt)),
            re_cp(np.ascontiguousarray(k[b].T).astype(dt)),
            re_st(np.ascontiguousarray(v[b].T).astype(dt)),
        )
    maps = []
    for c in range(8):
        b, g = c // 2, c % 2
        is_edge = g == 0 and b < 2
        rows = slice(g * DH, (g + 1) * DH)
        wq_c = np.ascontiguousarray(Wq[rows].T) * np.float32(1.0 / 8.0)
        bq_c = (bq[rows] * np.float32(1.0 / 8.0)).copy()
        if is_edge:
            wq_c[:, 0:DK] = 0.0
            bq_c[0:DK] = 0.0
        maps.append(
            {
                "eye": np.eye(P, dtype=dt),
                "wq": re_cp(wq_c.astype(dt)),
                "wk": re_cp(np.ascontiguousarray(Wk[rows].T).astype(dt)),
                "wv": re_cp(np.ascontiguousarray(Wv[rows].T).astype(dt)),
                "wo": re_cp(np.ascontiguousarray(Wo[:, rows].T).astype(dt)),
                "xq": xt[b][0],
                "xk": xt[b][1],
                "xv": xt[b][2],
                "bqk": np.concatenate([bq_c, bk[rows]]).reshape(2 * DH, 1),
                "edge": edge_t if is_edge else zeros_edge,
            }
        )
    return maps


def _ensure_ntff_hook():
    """Register the axon NTFF profile hook if the image's antenv lacks it."""
    import contextlib
    import ctypes
    import types

    try:
        from antenv.axon_hooks import get_axon_ntff_profile_hook  # noqa: F401
        return
    except ImportError:
        pass

    so_path = "/opt/axon/libaxon_pjrt.so"
    try:
        lib = ctypes.CDLL(so_path)
    except OSError:
        return
    if not hasattr(lib, "axon_start_nrt_profile"):
        return
    lib.axon_start_nrt_profile.argtypes = [
        ctypes.POINTER(ctypes.c_int64),
        ctypes.c_size_t,
    ]
    lib.axon_start_nrt_profile.restype = ctypes.c_int64
    lib.axon_stop_nrt_profile.argtypes = [ctypes.c_char_p]
    lib.axon_stop_nrt_profile.restype = ctypes.c_int64

    @contextlib.contextmanager
    def _hook(output_dir, device_ids):
        import jax

        jax.devices()
        if device_ids:
            ids = (ctypes.c_int64 * len(device_ids))(*device_ids)
            rc = lib.axon_start_nrt_profile(ids, len(device_ids))
        else:
            rc = lib.axon_start_nrt_profile(None, 0)
        if rc != 0:
            raise RuntimeError(f"axon_start_nrt_profile rc={rc}")
        try:
            yield
        finally:
            n = lib.axon_stop_nrt_profile(str(output_dir).encode())
            if n < 0:
                raise RuntimeError(f"axon_stop_nrt_profile rc={n}")

    _state = {"hook": _hook}
    mod = types.ModuleType("antenv.axon_hooks")
    mod.get_axon_ntff_profile_hook = lambda: _state["hook"]
    mod.set_axon_ntff_profile_hook = lambda h: _state.__setitem__("hook", h)
    import antenv

    antenv.axon_hooks = mod
    sys.modules["antenv.axon_hooks"] = mod


def kernel(q, k, v, edge_matrix, Wq, bq, Wk, bk, Wv, bv, Wo, bo, _trace=False):
    from concourse.bass_utils import run_bass_kernel_spmd

    if _trace:
        _ensure_ntff_hook()

    q, k, v = (np.asarray(t, np.float32) for t in (q, k, v))
    edge_matrix = np.asarray(edge_matrix, np.float32)
    Wq, bq, Wk, bk, Wv, bv, Wo, bo = (
        np.asarray(t, np.float32) for t in (Wq, bq, Wk, bk, Wv, bv, Wo, bo)
    )

    nc = _build()
    maps = _in_maps(q, k, v, edge_matrix, Wq, bq, Wk, bk, Wv, Wo)
    res = run_bass_kernel_spmd(nc, maps, core_ids=list(range(8)), trace=_trace)

    bo_eff = bo + Wo @ bv
    out = np.empty((B, SEQ, DO), np.float32)
    for b in range(B):
        out[b] = res.results[2 * b]["outp"] + res.results[2 * b + 1]["outp"] + bo_eff
    if _trace:
        return out, res
    return out



# revision 6
# speedup vs baseline: 1.0326x; 1.0326x over previous
"""Trainium2 Bass kernel for nn_MultiHeadAttention_5334349382389 (v3).

Sharding: 8 cores = 4 batches x 2 head-groups (4 heads each).
Core c handles batch b = c // 2, head-group g = c % 2 (heads 4g..4g+3).

Per-core math (fp16 matmuls, fp32 PSUM accumulate):
  qhT = (Wq_g/8) @ x_b^T + bq_g/8        [256, 1024]   (score scale folded into Wq)
  khT = Wk_g @ x_b^T + bk_g              [256, 1024]
  vh  = x_b @ Wv_g^T                     [1024, 256]   (bv folded into host-side bias)
  per head h: scoresT[k,q]; h==0 accumulates I @ edgeT on the PE into the
      score PSUM (edgeT is zeros on non-edge cores; Wq/bq head-0 slice
      zeroed on edge cores, so edge cores get scoresT == edgeT exactly)
  expT = exp(scoresT)                    (no max-subtraction; inputs bounded)
  outT_raw[d,q] accum over k-tiles with lhsT = [vh | ones] -> row 64 = denom
  OT = outT_raw[:64] * bcast(1/denom)
  partial = OT^T-contraction @ WoT_g     [1024, 512]
Host: out[b] = partial(b,0) + partial(b,1) + (bo + Wo @ bv).

v3 schedule (vs the 92.5us v2):
- dma_start only holds its engine ~0.7us (transfer is async); HBM ~360GB/s
  aggregate is the real limit, so DMA priority = per-queue FIFO order.
  Critical stream (twq/twk + xq both stripes + xk stripe 0) is split across
  sync/gpsimd/scalar and lands first; xv -> xk s1 -> edge -> wo trail.
- xq/xk packed stripe-major [P, NS, CD, STR] so projections chase the DMA
  per cd-chunk; first exp targeted ~12-13us (vs 27.4us).
- One flat software-pipelined loop over 32 (head, kt) steps, head order
  1,0,2,3. exp(step) emitted right after its score mms; PV(step) emitted
  two steps later (one step for kt7) so the PE never waits on the Act
  engine; PSUM: score ring bufs=2 (2 banks each), pv ring bufs=2, proj
  ring bufs=2 -> exactly 8 banks.
- PE filler work (v-projection pairs chasing xv, q-ch1/k-ch1 projections)
  is threaded into head-1 steps; deferred DVE hooks (biases, previous
  head's normalize) ride each step as in v2.
- Tail: no junk bridge; h3 normalize per stripe overlaps the output
  projection; oal casts rotate across DVE/gpsimd/scalar; stores rotate
  across the sync/gpsimd/scalar queues per m-tile.
"""

import os
import sys

sys.path.insert(0, "/opt/trn_rl_repo")

import numpy as np

B, SEQ, DIN, DO = 4, 1024, 512, 512
NH_ALL, DK = 8, 64
NHC = 4            # heads per core
DH = NHC * DK      # 256 per-core projected dims
P = 128
CD = DIN // P      # 4 contraction chunks for projections
CH = DH // P       # 2 dh chunks
KT = SEQ // P      # 8 k-tiles
STR = 512          # q-stripe (matmul free dim)
NS = SEQ // STR    # 2 stripes
TVW = NHC * (DK + 1) + DK - 1  # 323: per-k-tile aux width (4x65 + 63 pad)

NJUNK0 = int(os.environ.get("KERNEL_NJUNK0", "14"))

COMPUTE = os.environ.get("KERNEL_COMPUTE_DT", "fp16")  # fp16 | bf16 | fp32r

_nc = None


def _np_dt():
    import ml_dtypes

    return {
        "fp16": np.float16,
        "bf16": ml_dtypes.bfloat16,
        "fp32r": np.float32,
    }[COMPUTE]


def _build():
    global _nc
    if _nc is not None:
        return _nc
    import concourse.bacc as bacc
    import concourse.bass as bass
    import concourse.mybir as mybir
    import concourse.tile as tile

    f32 = mybir.dt.float32
    f32r = mybir.dt.float32r
    cdt = {
        "fp16": mybir.dt.float16,
        "bf16": mybir.dt.bfloat16,
        "fp32r": f32r,
    }[COMPUTE]
    Exp = mybir.ActivationFunctionType.Exp

    nc = bacc.Bacc("TRN2", target_bir_lowering=False, debug=False)

    eye_d = nc.dram_tensor("eye", (P, P), cdt, kind="ExternalInput")
    wq_d = nc.dram_tensor("wq", (P, CD * DH), cdt, kind="ExternalInput")
    wk_d = nc.dram_tensor("wk", (P, CD * DH), cdt, kind="ExternalInput")
    wv_d = nc.dram_tensor("wv", (P, CD * DH), cdt, kind="ExternalInput")
    wo_d = nc.dram_tensor("wo", (P, CH * DO), cdt, kind="ExternalInput")
    # xq/xk stripe-major: [P, NS, CD, STR]; xv k-tile-major: [P, KT, CD, P]
    xq_d = nc.dram_tensor("xq", (P, NS * CD * STR), cdt, kind="ExternalInput")
    xk_d = nc.dram_tensor("xk", (P, NS * CD * STR), cdt, kind="ExternalInput")
    xv_d = nc.dram_tensor("xv", (P, KT * CD * P), cdt, kind="ExternalInput")
    bqk = nc.dram_tensor("bqk", (2 * DH, 1), f32, kind="ExternalInput")
    edge = nc.dram_tensor("edge", (SEQ, SEQ), cdt, kind="ExternalInput")
    outp = nc.dram_tensor("outp", (SEQ, DO), cdt, kind="ExternalOutput")

    xq_r = xq_d.rearrange("p (s c n) -> p s c n", s=NS, c=CD)
    xk_r = xk_d.rearrange("p (s c n) -> p s c n", s=NS, c=CD)
    xv_r4 = xv_d.rearrange("p (t c j) -> p t c j", c=CD, j=P)
    edge_r2 = edge.rearrange("(t x p) n -> t p x n", x=2, p=P)
    out_r = outp.rearrange("(t p) n -> p t n", p=P)

    def sl(s):
        return slice(s * STR, (s + 1) * STR)

    with tile.TileContext(nc) as tc:
        with (
            tc.tile_pool(name="inp", bufs=1) as inp,
            tc.tile_pool(name="wts", bufs=1) as wts,
            tc.tile_pool(name="qkp", bufs=1) as qkp,
            tc.tile_pool(name="vhap", bufs=1) as vhap,
            tc.tile_pool(name="expp", bufs=8) as expp,
            tc.tile_pool(name="otp", bufs=1) as otp,
            tc.tile_pool(name="rrp", bufs=4) as rrp,
            tc.tile_pool(name="rbp", bufs=4) as rbp,
            tc.tile_pool(name="oalp", bufs=3) as oalp,
            tc.tile_pool(name="edgp", bufs=8) as edgp,
            # PSUM: 2*[P,SEQ] (4 banks) + 2*[P,STR] + 2*[P,STR] = 8 banks
            tc.tile_pool(name="bigp", bufs=2, space=bass.MemorySpace.PSUM) as bigp,
            tc.tile_pool(name="pvp", bufs=2, space=bass.MemorySpace.PSUM) as pvp,
            tc.tile_pool(name="prjp", bufs=2, space=bass.MemorySpace.PSUM) as prjp,
        ):
            # ---------------- tiles ----------------
            tjk = wts.tile([P, STR], cdt, tag="tjk")
            twq = wts.tile([P, CD, DH], cdt, tag="twq")
            twk = wts.tile([P, CD, DH], cdt, tag="twk")
            twv = wts.tile([P, CD, DH], cdt, tag="twv")
            two = wts.tile([P, CH, DO], cdt, tag="two")
            tb4 = wts.tile([P, 4, 1], f32, tag="tb4")
            teye = wts.tile([P, P], cdt, tag="teye")
            txq = inp.tile([P, NS, CD, STR], cdt, tag="txq")
            txk = inp.tile([P, NS, CD, STR], cdt, tag="txk")
            txv = inp.tile([P, KT, CD, P], cdt, tag="txv")
            tqh = qkp.tile([P, CH, SEQ], cdt, tag="tqh")
            khp = qkp.tile([P, NHC, SEQ], cdt, tag="khp")
            tvha = vhap.tile([P, KT, TVW], cdt, tag="tvha")
            tot = otp.tile([P, CH, SEQ], cdt, tag="tot")
            ed_pairs = [
                edgp.tile([P, 2, SEQ], cdt, tag="edg", name=f"edp{i}")
                for i in range(KT // 2)
            ]

            # ------- memsets: tjk on gpsimd (first op, gates junk); the rest
            # on DVE so the queues can start issuing DMAs immediately -------
            nc.gpsimd.memset(tjk, 0.0)
            # zero the unused partition-halves of khp (even heads: parts
            # 64-127, odd heads: parts 0-63) so score matmuls see zero weights
            nc.vector.memset(khp[0:DK, 1::2, :], 0.0)
            nc.vector.memset(khp[DK:P, 0::2, :], 0.0)
            # vh-aug tail pad + per-head ones columns (denominator rows)
            nc.vector.memset(tvha[:, :, NHC * (DK + 1) : TVW], 0.0)
            nc.vector.memset(
                tvha[:, :, 0 : NHC * (DK + 1)].rearrange(
                    "p t (h w) -> p t h w", w=DK + 1
                )[:, :, :, DK : DK + 1],
                1.0,
            )

            # ------- input DMAs.  Issue cost on the engine is ~0.7us; the
            # transfers drain asynchronously at HBM rate, so per-queue FIFO
            # order is the priority order.  scalar carries only small early
            # items and must be idle by the first exp (~12us). -------
            nc.scalar.dma_start(out=tb4, in_=bqk.rearrange("(c p) o -> p c o", p=P))
            nc.scalar.dma_start(
                out=twq, in_=wq_d.rearrange("p (c d) -> p c d", d=DH)
            )
            nc.scalar.dma_start(
                out=twk, in_=wk_d.rearrange("p (c d) -> p c d", d=DH)
            )
            nc.scalar.dma_start(out=teye, in_=eye_d[:, :])
            # critical stream, split for cd-chunk chasing
            nc.sync.dma_start(out=txq[:, 0, 0:2], in_=xq_r[:, 0, 0:2])
            nc.gpsimd.dma_start(out=txq[:, 0, 2:4], in_=xq_r[:, 0, 2:4])
            nc.sync.dma_start(out=txq[:, 1, 0:2], in_=xq_r[:, 1, 0:2])
            nc.gpsimd.dma_start(out=txq[:, 1, 2:4], in_=xq_r[:, 1, 2:4])
            nc.sync.dma_start(out=txk[:, 0, 0:2], in_=xk_r[:, 0, 0:2])
            nc.gpsimd.dma_start(out=txk[:, 0, 2:4], in_=xk_r[:, 0, 2:4])
            # second tier: v stream (vproj starts with h1), k stripe 1
            nc.sync.dma_start(out=txv[:, 0:2], in_=xv_r4[:, 0:2])
            nc.gpsimd.dma_start(out=txk[:, 1, 0:2], in_=xk_r[:, 1, 0:2])
            nc.sync.dma_start(out=txk[:, 1, 2:4], in_=xk_r[:, 1, 2:4])
            nc.gpsimd.dma_start(
                out=twv, in_=wv_d.rearrange("p (c d) -> p c d", d=DH)
            )
            nc.sync.dma_start(out=txv[:, 2:4], in_=xv_r4[:, 2:4])
            nc.gpsimd.dma_start(out=txv[:, 4:6], in_=xv_r4[:, 4:6])
            nc.sync.dma_start(out=txv[:, 6:8], in_=xv_r4[:, 6:8])
            # third tier: edge (head 0 runs second), wo (tail)
            nc.gpsimd.dma_start(out=ed_pairs[0], in_=edge_r2[0])
            nc.sync.dma_start(out=ed_pairs[1], in_=edge_r2[1])
            nc.gpsimd.dma_start(out=ed_pairs[2], in_=edge_r2[2])
            nc.sync.dma_start(out=ed_pairs[3], in_=edge_r2[3])
            nc.gpsimd.dma_start(
                out=two, in_=wo_d.rearrange("p (c d) -> p c d", d=DO)
            )

            # PE clock-ramp filler on the memset tile (no DMA dependency)
            def junk(n):
                jt = prjp.tile([P, STR], f32, tag="prj")
                for _ in range(n):
                    nc.tensor.matmul(
                        jt[:], lhsT=tjk[:, 0:P], rhs=tjk[:], start=True, stop=True
                    )

            junk(NJUNK0)

            # ------- critical projections: k-ch0 s0, q-ch0 (both stripes),
            # k-ch0 s1.  kc stripes ride the pvp ring (free until h1 PVs),
            # ptqa rides a bigp slot (freed by the tqh biases). -------
            kcA = pvp.tile([P, STR], f32, tag="pv")
            for cd in range(CD):
                nc.tensor.matmul(
                    kcA[:],
                    lhsT=twk[:, cd, 0:P],
                    rhs=txk[:, 0, cd, :],
                    start=(cd == 0),
                    stop=(cd == CD - 1),
                )
            ptqa = bigp.tile([P, SEQ], f32, tag="big")
            for cd in range(CD):
                nc.tensor.matmul(
                    ptqa[:, sl(0)],
                    lhsT=twq[:, cd, 0:P],
                    rhs=txq[:, 0, cd, :],
                    start=(cd == 0),
                    stop=(cd == CD - 1),
                )
            # biases for the first scores as soon as their psum lands
            nc.vector.tensor_scalar_add(
                out=khp[0:DK, 0, sl(0)], in0=kcA[0:DK, :], scalar1=tb4[0:DK, 2, :]
            )
            nc.vector.tensor_scalar_add(
                out=khp[DK:P, 1, sl(0)], in0=kcA[DK:P, :], scalar1=tb4[DK:P, 2, :]
            )
            nc.vector.tensor_scalar_add(
                out=tqh[:, 0, sl(0)], in0=ptqa[:, sl(0)], scalar1=tb4[:, 0, :]
            )
            for cd in range(CD):
                nc.tensor.matmul(
                    ptqa[:, sl(1)],
                    lhsT=twq[:, cd, 0:P],
                    rhs=txq[:, 1, cd, :],
                    start=(cd == 0),
                    stop=(cd == CD - 1),
                )
            nc.vector.tensor_scalar_add(
                out=tqh[:, 0, sl(1)], in0=ptqa[:, sl(1)], scalar1=tb4[:, 0, :]
            )
            kcB = pvp.tile([P, STR], f32, tag="pv")
            for cd in range(CD):
                nc.tensor.matmul(
                    kcB[:],
                    lhsT=twk[:, cd, 0:P],
                    rhs=txk[:, 1, cd, :],
                    start=(cd == 0),
                    stop=(cd == CD - 1),
                )
            nc.vector.tensor_scalar_add(
                out=khp[0:DK, 0, sl(1)], in0=kcB[0:DK, :], scalar1=tb4[0:DK, 2, :]
            )
            nc.vector.tensor_scalar_add(
                out=khp[DK:P, 1, sl(1)], in0=kcB[DK:P, :], scalar1=tb4[DK:P, 2, :]
            )

            # ---------------- flat software-pipelined main loop -----------
            # steps: (head, kt) in head order 1, 0, 2, 3.
            HEADS = (1, 0, 2, 3)
            steps = [(h, kt) for h in HEADS for kt in range(KT)]

            # PE fillers threaded into head-1 steps (index within head 1).
            # vproj pair j covers k-tiles 2j, 2j+1 (8 mms each); ch1
            # projections (ptqb = q-ch1, ptk2 = k-ch1) ride the prjp ring.
            def vproj_pair(j):
                vp = prjp.tile([P, STR], f32, tag="prj")
                for u in range(2):
                    for cd in range(CD):
                        nc.tensor.matmul(
                            vp[:, u * DH : (u + 1) * DH],
                            lhsT=txv[:, 2 * j + u, cd, :],
                            rhs=twv[:, cd, :],
                            start=(cd == 0),
                            stop=(cd == CD - 1),
                        )
                # copy into the augmented-vh layout (DVE)
                nc.vector.tensor_copy(
                    out=tvha[:, 2 * j : 2 * j + 2, 0 : NHC * (DK + 1)].rearrange(
                        "p t (h w) -> p t h w", w=DK + 1
                    )[:, :, :, 0:DK],
                    in_=vp[:].rearrange("p (t h d) -> p t h d", t=2, h=NHC),
                )

            def qch1_stripe(s):
                pt = prjp.tile([P, STR], f32, tag="prj")
                for cd in range(CD):
                    nc.tensor.matmul(
                        pt[:],
                        lhsT=twq[:, cd, P : 2 * P],
                        rhs=txq[:, s, cd, :],
                        start=(cd == 0),
                        stop=(cd == CD - 1),
                    )
                nc.vector.tensor_scalar_add(
                    out=tqh[:, 1, sl(s)], in0=pt[:], scalar1=tb4[:, 1, :]
                )

            def kch1_stripe(s):
                pt = prjp.tile([P, STR], f32, tag="prj")
                for cd in range(CD):
                    nc.tensor.matmul(
                        pt[:],
                        lhsT=twk[:, cd, P : 2 * P],
                        rhs=txk[:, s, cd, :],
                        start=(cd == 0),
                        stop=(cd == CD - 1),
                    )
                nc.vector.tensor_scalar_add(
                    out=khp[0:DK, 2, sl(s)], in0=pt[0:DK, :], scalar1=tb4[0:DK, 3, :]
                )
                nc.vector.tensor_scalar_add(
                    out=khp[DK:P, 3, sl(s)], in0=pt[DK:P, :], scalar1=tb4[DK:P, 3, :]
                )

            fillers = {
                (1, 0): lambda: vproj_pair(0),
                (1, 1): lambda: qch1_stripe(0),
                (1, 2): lambda: vproj_pair(1),
                (1, 3): lambda: qch1_stripe(1),
                (1, 4): lambda: vproj_pair(2),
                (1, 5): lambda: kch1_stripe(0),
                (1, 6): lambda: vproj_pair(3),
                (1, 7): lambda: kch1_stripe(1),
            }

            # deferred normalize for the previous head, hooked into the
            # next head's early steps (kt 1 and 2; PV of kt7 lands at kt0)
            def norm_stripe(h, pvs, s):
                rr = rrp.tile([1, STR], f32, tag="rr")
                rs = rrp.tile([1, STR], f32, tag="rs")
                nc.vector.tensor_copy(out=rs[:], in_=pvs[s][DK : DK + 1, :])
                nc.vector.reciprocal_approx_fast(out=rr[:], in_=rs[:])
                rb = rbp.tile([DK, STR], f32, tag="rb")
                nc.gpsimd.partition_broadcast(rb[:], rr[:])
                ch, off = h // 2, (h % 2) * DK
                nc.vector.tensor_mul(
                    tot[off : off + DK, ch, sl(s)], pvs[s][0:DK, :], rb[:]
                )

            # main loop state
            pv_by_head = {}
            te_by_step = {}
            stt_ring = {}
            prev_head = {1: None, 0: 1, 2: 0, 3: 2}

            def emit_pv(h, kt, stop):
                pvs = pv_by_head[h]
                te = te_by_step[(h, kt)]
                for s in range(NS):
                    nc.tensor.matmul(
                        pvs[s][:],
                        lhsT=tvha[:, kt, h * (DK + 1) : h * (DK + 1) + P],
                        rhs=te[:, sl(s)],
                        start=(kt == 0),
                        stop=stop,
                    )

            for i, (h, kt) in enumerate(steps):
                ch = h // 2
                # allocate this head's pv tiles at its first step
                if kt == 0:
                    pv_by_head[h] = (
                        pvp.tile([P, STR], f32, tag="pv", name=f"pv{h}s0"),
                        pvp.tile([P, STR], f32, tag="pv", name=f"pv{h}s1"),
                    )
                # scores for (h, kt); head 0 accumulates I @ edgeT on top
                stt = bigp.tile([P, SEQ], f32, tag="big")
                for s in range(NS):
                    nc.tensor.matmul(
                        stt[:, sl(s)],
                        lhsT=khp[:, h, kt * P : (kt + 1) * P],
                        rhs=tqh[:, ch, sl(s)],
                        start=True,
                        stop=(h != 0),
                    )
                    if h == 0:
                        nc.tensor.matmul(
                            stt[:, sl(s)],
                            lhsT=teye[:],
                            rhs=ed_pairs[kt // 2][:, kt % 2, sl(s)],
                            start=False,
                            stop=True,
                        )
                # exp on the Act engine
                te = expp.tile([P, SEQ], cdt, tag="expT")
                nc.scalar.activation(out=te, in_=stt[:], func=Exp)
                te_by_step[(h, kt)] = te
                # PE fillers for this step (head 1 only)
                f = fillers.get((h, kt))
                if f is not None:
                    f()
                # previous head's normalize hooks (frees the pvp ring)
                ph = prev_head[h]
                if ph is not None and kt == 1:
                    norm_stripe(ph, pv_by_head[ph], 0)
                if ph is not None and kt == 2:
                    norm_stripe(ph, pv_by_head[ph], 1)
                # lagged PV matmuls: step i-2, plus kt7 of the previous
                # head at the next head's kt0 (lag 1)
                if kt == 0 and ph is not None:
                    emit_pv(ph, KT - 2, stop=False)
                    emit_pv(ph, KT - 1, stop=True)
                elif kt >= 2:
                    emit_pv(h, kt - 2, stop=False)

            # ---------------- tail ----------------
            h_last = HEADS[-1]
            emit_pv(h_last, KT - 2, stop=False)
            junk(3)
            emit_pv(h_last, KT - 1, stop=True)

            # h3 normalize, per stripe; stripe 0 gates out-proj m 0-3.
            # denominator copy on the Act engine (idle after the last exp)
            pvs3 = pv_by_head[h_last]
            ch3, off3 = h_last // 2, (h_last % 2) * DK

            def norm_tail(s):
                rs = rrp.tile([1, STR], f32, tag="rs")
                nc.scalar.copy(out=rs[:], in_=pvs3[s][DK : DK + 1, :])
                rr = rrp.tile([1, STR], f32, tag="rr")
                nc.vector.reciprocal_approx_fast(out=rr[:], in_=rs[:])
                rb = rbp.tile([DK, STR], f32, tag="rb")
                nc.gpsimd.partition_broadcast(rb[:], rr[:])
                nc.vector.tensor_mul(
                    tot[off3 : off3 + DK, ch3, sl(s)], pvs3[s][0:DK, :], rb[:]
                )

            norm_tail(0)
            junk(4)
            norm_tail(1)

            # output projection, per m-tile; stripe 0 (m 0-3) first.
            cast_ops = [
                lambda o, i: nc.vector.tensor_copy(out=o, in_=i),
                lambda o, i: nc.scalar.copy(out=o, in_=i),
            ]
            store_eng = [nc.sync, nc.gpsimd, nc.scalar]
            for m in range(KT):
                po = bigp.tile([P, SEQ], f32, tag="big")
                for ci in range(CH):
                    nc.tensor.matmul(
                        po[:, 0:DO],
                        lhsT=tot[:, ci, m * P : (m + 1) * P],
                        rhs=two[:, ci, :],
                        start=(ci == 0),
                        stop=(ci == CH - 1),
                    )
                oal = oalp.tile([P, DO], cdt, tag="oall")
                cast_ops[m % 2](oal[:], po[:, 0:DO])
                store_eng[m % 3].dma_start(out=out_r[:, m], in_=oal[:])

    nc.compile()
    _nc = nc
    return nc


def _in_maps(q, k, v, edge_matrix, Wq, bq, Wk, bk, Wv, Wo):
    dt = _np_dt()
    zeros_edge = np.zeros((SEQ, SEQ), dt)
    edge_t = np.ascontiguousarray(edge_matrix.T).astype(dt)

    def re_cp(m):
        # [C*P, D] -> [P, C*D] (partition-major packing of "(c p) d -> p c d")
        cp, d = m.shape
        return np.ascontiguousarray(
            m.reshape(cp // P, P, d).transpose(1, 0, 2).reshape(P, -1)
        )

    def re_sp(m):
        # [CD*P, NS*STR] -> [P, NS*CD*STR] stripe-major packing
        return np.ascontiguousarray(
            m.reshape(CD, P, NS, STR).transpose(1, 2, 0, 3).reshape(P, -1)
        )

    def re_st(m):
        # [CD*P, KT*P] -> [P, KT*CD*P]: kt-major packing for the v stream
        return np.ascontiguousarray(
            m.reshape(CD, P, KT, P).transpose(1, 2, 0, 3).reshape(P, -1)
        )

    xt = {}
    for b in range(B):
        xt[b] = (
            re_sp(np.ascontiguousarray(q[b].T).astype(dt)),
            re_sp(np.ascontiguousarray(k[b].T).astype(dt)),
            re_st(np.ascontiguousarray(v[b].T).astype(dt)),
        )
    maps = []
    for c in range(8):
        b, g = c // 2, c % 2
        is_edge = g == 0 and b < 2
        rows = slice(g * DH, (g + 1) * DH)
        wq_c = np.ascontiguousarray(Wq[rows].T) * np.float32(1.0 / 8.0)
        bq_c = (bq[rows] * np.float32(1.0 / 8.0)).copy()
        if is_edge:
            wq_c[:, 0:DK] = 0.0
            bq_c[0:DK] = 0.0
        maps.append(
            {
                "eye": np.eye(P, dtype=dt),
                "wq": re_cp(wq_c.astype(dt)),
                "wk": re_cp(np.ascontiguousarray(Wk[rows].T).astype(dt)),
                "wv": re_cp(np.ascontiguousarray(Wv[rows].T).astype(dt)),
                "wo": re_cp(np.ascontiguousarray(Wo[:, rows].T).astype(dt)),
                "xq": xt[b][0],
                "xk": xt[b][1],
                "xv": xt[b][2],
                "bqk": np.concatenate([bq_c, bk[rows]]).reshape(2 * DH, 1),
                "edge": edge_t if is_edge else zeros_edge,
            }
        )
    return maps


def _ensure_ntff_hook():
    """Register the axon NTFF profile hook if the image's antenv lacks it."""
    import contextlib
    import ctypes
    import types

    try:
        from antenv.axon_hooks import get_axon_ntff_profile_hook  # noqa: F401
        return
    except ImportError:
        pass

    so_path = "/opt/axon/libaxon_pjrt.so"
    try:
        lib = ctypes.CDLL(so_path)
    except OSError:
        return
    if not hasattr(lib, "axon_start_nrt_profile"):
        return
    lib.axon_start_nrt_profile.argtypes = [
        ctypes.POINTER(ctypes.c_int64),
        ctypes.c_size_t,
    ]
    lib.axon_start_nrt_profile.restype = ctypes.c_int64
    lib.axon_stop_nrt_profile.argtypes = [ctypes.c_char_p]
    lib.axon_stop_nrt_profile.restype = ctypes.c_int64

    @contextlib.contextmanager
    def _hook(output_dir, device_ids):
        import jax

        jax.devices()
        if device_ids:
            ids = (ctypes.c_int64 * len(device_ids))(*device_ids)
            rc = lib.axon_start_nrt_profile(ids, len(device_ids))
        else:
            rc = lib.axon_start_nrt_profile(None, 0)
        if rc != 0:
            raise RuntimeError(f"axon_start_nrt_profile rc={rc}")
        try:
            yield
        finally:
            n = lib.axon_stop_nrt_profile(str(output_dir).encode())
            if n < 0:
                raise RuntimeError(f"axon_stop_nrt_profile rc={n}")

    _state = {"hook": _hook}
    mod = types.ModuleType("antenv.axon_hooks")
    mod.get_axon_ntff_profile_hook = lambda: _state["hook"]
    mod.set_axon_ntff_profile_hook = lambda h: _state.__setitem__("hook", h)
    import antenv

    antenv.axon_hooks = mod
    sys.modules["antenv.axon_hooks"] = mod


def kernel(q, k, v, edge_matrix, Wq, bq, Wk, bk, Wv, bv, Wo, bo, _trace=False):
    from concourse.bass_utils import run_bass_kernel_spmd

    if _trace:
        _ensure_ntff_hook()

    q, k, v = (np.asarray(t, np.float32) for t in (q, k, v))
    edge_matrix = np.asarray(edge_matrix, np.float32)
    Wq, bq, Wk, bk, Wv, bv, Wo, bo = (
        np.asarray(t, np.float32) for t in (Wq, bq, Wk, bk, Wv, bv, Wo, bo)
    )

    nc = _build()
    maps = _in_maps(q, k, v, edge_matrix, Wq, bq, Wk, bk, Wv, Wo)
    res = run_bass_kernel_spmd(nc, maps, core_ids=list(range(8)), trace=_trace)

    bo_eff = bo + Wo @ bv
    out = np.empty((B, SEQ, DO), np.float32)
    for b in range(B):
        out[b] = res.results[2 * b]["outp"] + res.results[2 * b + 1]["outp"] + bo_eff
    if _trace:
        return out, res
    return out
